# revision 8
# baseline (speedup 1.0000x reference)
"""Locally-connected conv (per-pixel weights, 3x3, same-pad) + ReLU on 8 TRN2 cores.

Math: out[b, co, h, w] = relu( sum_{ci,a,e} W[h, w, co, ci, a, e] * xpad[b, ci, h+a, w+e] )
Shapes: x [16, 32, 64, 64] f32, W [64, 64, 32, 32, 3, 3] f32, out [16, 32, 64, 64] f32.

Sharding: data-parallel over h (8 rows/core) with a 1-row halo on x; each core
gets its weight h-slice (the 151MB weight tensor dominates: ~18.9MB/core).

Per-core algorithm (pixel-group matmul, v2):
  - pixels grouped 4-at-a-time along w CONSECUTIVELY: group j = {4j..4j+3}
  - weights DMA'd dense as ONE [128=(g,co) partitions, 16x288=(j,ci,a,e)] DMA
    per h-row (2.36MB, full-partition, near peak HBM BW)
  - reorder+cast (ci,a,e)->(a,(e,ci)) bf16 on GPSIMD (otherwise-idle engine,
    so DVE/ACT FIFOs never gate the PE-feeding chain)
  - TensorE identity-transpose per (j,a): [128,96] -> TW [96=(e,ci), 128=(g,co)]
  - matmul rhs is a direct strided AP into x_rep16 [96=(e,ci), b, h, w]
    (consecutive grouping makes the 4 pixel windows an affine AP: no x shuffle)
  - 3 accumulating matmuls per group -> PSUM [128=(g,co), 64=(g',b)]; diagonal
    g'==g blocks are the outputs
  - ReLU + diagonal extraction on ScalarE into outT [32=co, b, r, w]
  - ONE output DMA at the end (256B runs), issued from ACT so the SP DMA FIFO
    carries only weight loads (keeps weight prefetch streaming across reps)
"""

import sys

import numpy as np

for _p in ("/opt/trn_rl_repo", "/root/.axon_site/_ro/trn_rl_repo"):
    if _p not in sys.path:
        sys.path.append(_p)

import concourse.bass as bass
import concourse.mybir as mybir
import concourse.tile as tile
from concourse.vector_clock import ScopedClock
from concourse.bass_utils import run_bass_kernel_spmd

B, CIN, COUT, H, W, K = 16, 32, 32, 64, 64, 3
NCORES = 8
HC = H // NCORES          # h rows per core
HH = HC + 2               # with halo
WP = W + 2                # w padded
KK = CIN * K * K          # 288 contraction
NG = W // 4               # 16 groups per row
NGH = NG // 2             # groups per half-row (PSUM bank sizing)
F32 = mybir.dt.float32
BF16 = mybir.dt.bfloat16


class PatchedTileContext(tile.TileContext):
    """This walrus build supports one sem-wait per instruction; the stock
    tile-exit drain aggregates one wait per DMA-queue proc. Spread the extra
    waits over dedicated SP nop carriers."""

    def _drain_and_barrier(self, tick_clock, wait_clock):
        nc = self.nc
        drain_inst = nc.sync.drain()
        wait_clock.add_sem_waits(
            drain_inst.ins, ScopedClock({None: tick_clock.global_clock})
        )
        si = drain_inst.ins.sync_info
        if si is not None and len(si.on_wait) > 1:
            waits = list(si.on_wait)
            upds = list(si.on_update)
            drain_inst.ins.sync_info = mybir.SyncInfo(
                on_wait=[waits[0]], on_update=upds
            )
            for w in waits[1:]:
                n = nc.sync.nop()
                n.ins.sync_info = mybir.SyncInfo(on_wait=[w], on_update=[])
        nc.all_engine_barrier()
        popped = nc._tile_sem_poison_stack.pop()
        assert popped is self._sem_poison
        nc.clear_and_free_semaphores(list(self.sems.allocated().values()))
        nc.all_engine_barrier()


def _split_multi_waits(nc):
    """This walrus build rejects >1 sem-wait per instruction. Hoist extra waits
    onto same-engine NoOp carriers inserted right before the offender."""
    ctr = 0
    for f in nc.m.functions:
        for bb in f.blocks:
            new = []
            for inst in bb.instructions:
                si = inst.sync_info
                if si is not None and len(si.on_wait) > 1:
                    waits = list(si.on_wait)
                    upds = list(si.on_update)
                    for w in waits[:-1]:
                        n = mybir.InstNoOp(name=f"zwaitcar-{ctr}", ins=[], outs=[])
                        ctr += 1
                        n.engine = inst.engine
                        n.sync_info = mybir.SyncInfo(on_wait=[w], on_update=[])
                        nc.register_instruction(n, overwrite=True)
                        new.append(n)
                    inst.sync_info = mybir.SyncInfo(
                        on_wait=[waits[-1]], on_update=upds
                    )
                new.append(inst)
            bb.instructions = new


def _build_nc(reps: int = 1):
    import os

    ablate = set(os.environ.get("ABLATE", "").split(","))
    nc = bass.Bass("TRN2")
    xs = nc.dram_tensor("xs", [B, CIN, HH, WP], F32, kind="ExternalInput")
    ws = nc.dram_tensor("ws", [HC, W, COUT, CIN, K, K], F32, kind="ExternalInput")
    ident = nc.dram_tensor("ident", [128, 128], F32, kind="ExternalInput")
    out = nc.dram_tensor("out", [B, COUT, HC, W], F32, kind="ExternalOutput")

    # DRAM views
    # weights: w-pixel = 4*j + g  ->  partitions (g, co), free (j, kk)
    wv = ws.rearrange("r (j g) co ci a e -> r (g co) j (ci a e)", g=4)
    # x: partition = ci
    xv = xs.rearrange("b ci h w -> ci b h w")
    # out: [co, b, r, w] (per-partition 256B w-runs)
    ov = out.rearrange("b co r w -> co b r w")

    with PatchedTileContext(nc) as tc:
        with (
            tc.tile_pool(name="singles", bufs=1) as singles,
            tc.tile_pool(name="wrow", bufs=3) as wrow_pool,
            tc.tile_pool(name="wrowR", bufs=3) as wrowR_pool,
            tc.tile_pool(name="tw", bufs=6) as tw_pool,
            tc.tile_pool(name="psumT", bufs=6, space="PSUM") as psumT_pool,
            tc.tile_pool(name="psumO", bufs=2, space="PSUM") as psumO_pool,
        ):
            # --- one-time setup ---
            id_sb = singles.tile([128, 128], F32)
            nc.sync.dma_start(out=id_sb[:], in_=ident[:])
            id16 = singles.tile([128, 128], BF16)
            nc.vector.tensor_copy(id16[:], id_sb[:])

            # x0 [32=ci, b, h, wp] f32: one 1.35MB DMA (2640B runs)
            x0 = singles.tile([CIN, B, HH, WP], F32)
            nc.sync.dma_start(out=x0[:], in_=xv[:])

            # x_rep16 [96=(e,ci), b, h, w]: value = xpad[b, ci, h, w+e], bf16.
            # 9 cast-copies (3 e-shifts x 3 b-ranges) split by engine rate.
            x_rep16 = singles.tile([96, B, HH, W], BF16)
            bsplit = [(nc.vector, 0, 7), (nc.scalar, 7, 12), (nc.gpsimd, 12, 16)]
            for e in range(K):
                for eng, b0, b1 in bsplit:
                    dst = x_rep16[32 * e : 32 * e + 32, b0:b1]
                    src = x0[:, b0:b1, :, e : e + W]
                    if eng is nc.scalar:
                        eng.copy(dst, src)
                    else:
                        eng.tensor_copy(dst, src)

            # outT [32=co, b, r, w] f32 staging for the single output DMA
            outT = singles.tile([COUT, B, HC, W], F32)

            # --- main loop over the 8 h-rows (optionally repeated for timing) ---
            rep_ctx = tc.For_i(0, reps, 1) if reps > 1 else None
            if rep_ctx is not None:
                rep_ctx.__enter__()
            for r in range(HC):
                # KK+1 per group so the reorder can read a 4-wide e-window
                # (4th lane = pad landing in transpose cols 96:128, never
                # consumed by matmuls; memset keeps element 288 finite).
                wrow = wrow_pool.tile([128, NG, KK + 1], F32)
                nc.vector.memset(wrow[:, :, KK], 0.0)
                if "nodma_w" not in ablate:
                    nc.sync.dma_start(out=wrow[:, :, 0:KK], in_=wv[r])
                # reorder k=(ci,a,e) -> (a,(e',ci)), e' 4-wide overlapping
                # window + cast bf16 on GPSIMD: transpose inputs get 128 cols
                # (FWL-eligible stationary loads)
                wrowR = wrowR_pool.tile([128, NG, K, 128], BF16)
                for j in range(NG):
                    src = wrow[:, j, 0:KK].rearrange(
                        "p (a ep ci) -> p a ep ci", a=K, ep=4
                    )
                    src.ap[1] = (3, K)    # a:  stride 3, size 3
                    src.ap[2] = (1, 4)    # e': stride 1, size 4 (overlaps)
                    src.ap[3] = (9, CIN)  # ci: stride 9, size 32
                    dst = wrowR[:, j].rearrange("p a (ep ci) -> p a ep ci", ep=4)
                    nc.gpsimd.tensor_copy(dst, src)

                def emit_transpose_and_copy(jj):
                    tw = tw_pool.tile([96, K, 128], BF16, tag="tw")
                    if "notrans" in ablate:
                        return tw
                    psts = []
                    for a in range(K):
                        pst = psumT_pool.tile([128, 128], BF16, tag="pst")
                        nc.tensor.transpose(pst[:], wrowR[:, jj, a], id16[:])
                        psts.append(pst)
                    for a in range(K):
                        if (jj + a) % 2 == 0:
                            nc.vector.tensor_copy(tw[:, a], psts[a][0:96, :])
                        else:
                            nc.scalar.copy(tw[:, a], psts[a][0:96, :])
                    return tw

                # outT w-view for this row: (half, g, j) -> w = 32*half + 4*j + g
                ot = outT[:, :, r, :].rearrange(
                    "co b (hf j g) -> co hf g j b", hf=2, g=4
                )

                # software pipeline: transposes run one group ahead of matmuls
                tws = {0: emit_transpose_and_copy(0)}
                for half in range(2):
                    po = psumO_pool.tile([128, NGH, 4, B], F32, tag="po")
                    for jh in range(NGH):
                        j = half * NGH + jh
                        if j + 1 < NG:
                            tws[j + 1] = emit_transpose_and_copy(j + 1)
                        tw = tws.pop(j)
                        if "nomm" not in ablate:
                            for a in range(K):
                                rhs = x_rep16[
                                    :, :, r + a, 4 * j : 4 * j + 4
                                ].rearrange("p b g -> p g b")
                                nc.tensor.matmul(
                                    po[:, jh],
                                    tw[:, a],
                                    rhs,
                                    start=(a == 0),
                                    stop=(a == K - 1),
                                )
                        elif jh == 0:
                            nc.vector.memset(po[:, jh], 0.0)

                    # ReLU + extract diagonal blocks (g' == g) for the half-row
                    for g in range(4):
                        nc.scalar.activation(
                            ot[:, half, g],
                            po[32 * g : 32 * g + 32, :, g, :],
                            mybir.ActivationFunctionType.Relu,
                        )

            # single output DMA, issued from ACT (keeps SP FIFO = weight DMAs)
            nc.scalar.dma_start(out=ov[:], in_=outT[:])
            if rep_ctx is not None:
                rep_ctx.__exit__(None, None, None)
    _split_multi_waits(nc)
    return nc


_NC_CACHE = None


def _in_maps(x: np.ndarray, weights: np.ndarray):
    xp = np.pad(x, ((0, 0), (0, 0), (1, 1), (1, 1)))  # [B, CIN, H+2, W+2]
    ident = np.eye(128, dtype=np.float32)
    maps = []
    for c in range(NCORES):
        h0 = c * HC
        maps.append(
            {
                "xs": np.ascontiguousarray(xp[:, :, h0 : h0 + HH, :]),
                "ws": np.ascontiguousarray(weights[h0 : h0 + HC]),
                "ident": ident,
            }
        )
    return maps


def kernel(x: np.ndarray, weights: np.ndarray) -> np.ndarray:
    global _NC_CACHE
    x = np.ascontiguousarray(x, dtype=np.float32)
    weights = np.ascontiguousarray(weights, dtype=np.float32)
    if _NC_CACHE is None:
        _NC_CACHE = _build_nc()
    res = run_bass_kernel_spmd(
        _NC_CACHE, _in_maps(x, weights), core_ids=list(range(NCORES))
    )
    out = np.concatenate([res.results[c]["out"] for c in range(NCORES)], axis=2)
    return np.ascontiguousarray(out, dtype=np.float32)


if __name__ == "__main__":
    rng = np.random.default_rng(0)
    x = rng.standard_normal((B, CIN, H, W), dtype=np.float32)
    w = rng.standard_normal((H, W, COUT, CIN, K, K), dtype=np.float32) / CIN
    y = kernel(x, w)
    print("out shape", y.shape, y.dtype)


# revision 10
# speedup vs baseline: 1.4781x; 1.4781x over previous
"""Locally-connected conv (per-pixel weights, 3x3, same-pad) + ReLU on 8 TRN2 cores.

Math: out[b, co, h, w] = relu( sum_{ci,a,e} W[h, w, co, ci, a, e] * xpad[b, ci, h+a, w+e] )
Shapes: x [16, 32, 64, 64] f32, W [64, 64, 32, 32, 3, 3] f32, out [16, 32, 64, 64] f32.

Sharding: data-parallel over h (8 rows/core) with a 1-row halo on x; each core
gets its weight h-slice (the 151MB weight tensor dominates: ~18.9MB/core).

Per-core algorithm (pixel-group matmul, v2):
  - pixels grouped 4-at-a-time along w CONSECUTIVELY: group j = {4j..4j+3}
  - weights DMA'd dense as ONE [128=(g,co) partitions, 16x288=(j,ci,a,e)] DMA
    per h-row (2.36MB, full-partition, near peak HBM BW)
  - reorder+cast (ci,a,e)->(a,(e,ci)) bf16 on GPSIMD (otherwise-idle engine,
    so DVE/ACT FIFOs never gate the PE-feeding chain)
  - TensorE identity-transpose per (j,a): [128,96] -> TW [96=(e,ci), 128=(g,co)]
  - matmul rhs is a direct strided AP into x_rep16 [96=(e,ci), b, h, w]
    (consecutive grouping makes the 4 pixel windows an affine AP: no x shuffle)
  - 3 accumulating matmuls per group -> PSUM [128=(g,co), 64=(g',b)]; diagonal
    g'==g blocks are the outputs
  - ReLU + diagonal extraction on ScalarE into outT [32=co, b, r, w]
  - ONE output DMA at the end (256B runs), issued from ACT so the SP DMA FIFO
    carries only weight loads (keeps weight prefetch streaming across reps)
"""

import sys

import numpy as np

for _p in ("/opt/trn_rl_repo", "/root/.axon_site/_ro/trn_rl_repo"):
    if _p not in sys.path:
        sys.path.append(_p)

import concourse.bass as bass
import concourse.mybir as mybir
import concourse.tile as tile
from concourse.vector_clock import ScopedClock
from concourse.bass_utils import run_bass_kernel_spmd

B, CIN, COUT, H, W, K = 16, 32, 32, 64, 64, 3
NCORES = 8
HC = H // NCORES          # h rows per core
HH = HC + 2               # with halo
WP = W + 2                # w padded
KK = CIN * K * K          # 288 contraction
NG = W // 4               # 16 groups per row
NGH = NG // 2             # groups per half-row (PSUM bank sizing)
F32 = mybir.dt.float32
BF16 = mybir.dt.bfloat16


class PatchedTileContext(tile.TileContext):
    """This walrus build supports one sem-wait per instruction; the stock
    tile-exit drain aggregates one wait per DMA-queue proc. Spread the extra
    waits over dedicated SP nop carriers."""

    def _drain_and_barrier(self, tick_clock, wait_clock):
        nc = self.nc
        drain_inst = nc.sync.drain()
        wait_clock.add_sem_waits(
            drain_inst.ins, ScopedClock({None: tick_clock.global_clock})
        )
        si = drain_inst.ins.sync_info
        if si is not None and len(si.on_wait) > 1:
            waits = list(si.on_wait)
            upds = list(si.on_update)
            drain_inst.ins.sync_info = mybir.SyncInfo(
                on_wait=[waits[0]], on_update=upds
            )
            for w in waits[1:]:
                n = nc.sync.nop()
                n.ins.sync_info = mybir.SyncInfo(on_wait=[w], on_update=[])
        nc.all_engine_barrier()
        popped = nc._tile_sem_poison_stack.pop()
        assert popped is self._sem_poison
        nc.clear_and_free_semaphores(list(self.sems.allocated().values()))
        nc.all_engine_barrier()


def _split_multi_waits(nc):
    """This walrus build rejects >1 sem-wait per instruction. Hoist extra waits
    onto same-engine NoOp carriers inserted right before the offender."""
    ctr = 0
    for f in nc.m.functions:
        for bb in f.blocks:
            new = []
            for inst in bb.instructions:
                si = inst.sync_info
                if si is not None and len(si.on_wait) > 1:
                    waits = list(si.on_wait)
                    upds = list(si.on_update)
                    for w in waits[:-1]:
                        n = mybir.InstNoOp(name=f"zwaitcar-{ctr}", ins=[], outs=[])
                        ctr += 1
                        n.engine = inst.engine
                        n.sync_info = mybir.SyncInfo(on_wait=[w], on_update=[])
                        nc.register_instruction(n, overwrite=True)
                        new.append(n)
                    inst.sync_info = mybir.SyncInfo(
                        on_wait=[waits[-1]], on_update=upds
                    )
                new.append(inst)
            bb.instructions = new


def _build_nc(reps: int = 1):
    import os

    ablate = set(os.environ.get("ABLATE", "").split(","))
    nc = bass.Bass("TRN2")
    xs = nc.dram_tensor("xs", [B, CIN, HH, WP], F32, kind="ExternalInput")
    ws = nc.dram_tensor("ws", [HC, W, COUT, CIN, K, K], F32, kind="ExternalInput")
    ident = nc.dram_tensor("ident", [128, 128], F32, kind="ExternalInput")
    out = nc.dram_tensor("out", [B, COUT, HC, W], F32, kind="ExternalOutput")

    # DRAM views
    # weights: w-pixel = 4*j + g  ->  partitions (g, co), free (j, kk)
    wv = ws.rearrange("r (j g) co ci a e -> r (g co) j (ci a e)", g=4)
    # x: partition = ci
    xv = xs.rearrange("b ci h w -> ci b h w")
    # out: [co, b, r, w] (per-partition 256B w-runs)
    ov = out.rearrange("b co r w -> co b r w")

    with PatchedTileContext(nc) as tc:
        with (
            tc.tile_pool(name="singles", bufs=1) as singles,
            tc.tile_pool(name="wrow", bufs=3) as wrow_pool,
            tc.tile_pool(name="wrowR", bufs=3) as wrowR_pool,
            tc.tile_pool(name="tw", bufs=6) as tw_pool,
            tc.tile_pool(name="psumT", bufs=6, space="PSUM") as psumT_pool,
            tc.tile_pool(name="psumO", bufs=2, space="PSUM") as psumO_pool,
        ):
            # --- one-time setup ---
            id_sb = singles.tile([128, 128], F32)
            nc.sync.dma_start(out=id_sb[:], in_=ident[:])
            id16 = singles.tile([128, 128], BF16)
            nc.vector.tensor_copy(id16[:], id_sb[:])

            # x0 [32=ci, b, h, wp] f32: one 1.35MB DMA (2640B runs)
            x0 = singles.tile([CIN, B, HH, WP], F32)
            nc.sync.dma_start(out=x0[:], in_=xv[:])

            # x_rep16 [96=(e,ci), b, h, w]: value = xpad[b, ci, h, w+e], bf16.
            # 9 cast-copies (3 e-shifts x 3 b-ranges) split by engine rate.
            x_rep16 = singles.tile([96, B, HH, W], BF16)
            bsplit = [(nc.vector, 0, 7), (nc.scalar, 7, 12), (nc.gpsimd, 12, 16)]
            for e in range(K):
                for eng, b0, b1 in bsplit:
                    dst = x_rep16[32 * e : 32 * e + 32, b0:b1]
                    src = x0[:, b0:b1, :, e : e + W]
                    if eng is nc.scalar:
                        eng.copy(dst, src)
                    else:
                        eng.tensor_copy(dst, src)

            # outT [32=co, b, r, w] f32 staging for the single output DMA
            outT = singles.tile([COUT, B, HC, W], F32)

            # --- main loop over the 8 h-rows (optionally repeated for timing) ---
            rep_ctx = tc.For_i(0, reps, 1) if reps > 1 else None
            if rep_ctx is not None:
                rep_ctx.__enter__()
            for r in range(HC):
                wrow = wrow_pool.tile([128, NG, KK], F32)
                if "nodma_w" not in ablate:
                    nc.sync.dma_start(out=wrow[:], in_=wv[r])
                wrow4 = wrow.rearrange("p j (ci a e) -> p j ci a e", ci=CIN, a=K)
                # reorder k=(ci,a,e) -> (a,(e,ci)) + cast bf16 on GPSIMD
                wrowR = wrowR_pool.tile([128, NG, K, 96], BF16)
                for j in range(NG):
                    nc.gpsimd.tensor_copy(
                        wrowR[:, j], wrow4[:, j].rearrange("p ci a e -> p a e ci")
                    )

                def emit_transpose_and_copy(jj):
                    tw = tw_pool.tile([96, K, 128], BF16, tag="tw")
                    if "notrans" in ablate:
                        return tw
                    psts = []
                    for a in range(K):
                        pst = psumT_pool.tile([96, 128], BF16, tag="pst")
                        nc.tensor.transpose(pst[:], wrowR[:, jj, a], id16[:])
                        psts.append(pst)
                    for a in range(K):
                        if (jj + a) % 2 == 0:
                            nc.vector.tensor_copy(tw[:, a], psts[a][:])
                        else:
                            nc.scalar.copy(tw[:, a], psts[a][:])
                    return tw

                # outT w-view for this row: (half, g, j) -> w = 32*half + 4*j + g
                ot = outT[:, :, r, :].rearrange(
                    "co b (hf j g) -> co hf g j b", hf=2, g=4
                )

                # software pipeline: transposes run one group ahead of matmuls
                tws = {0: emit_transpose_and_copy(0)}
                for half in range(2):
                    po = psumO_pool.tile([128, NGH, 4, B], F32, tag="po")
                    for jh in range(NGH):
                        j = half * NGH + jh
                        if j + 1 < NG:
                            tws[j + 1] = emit_transpose_and_copy(j + 1)
                        tw = tws.pop(j)
                        if "nomm" not in ablate:
                            for a in range(K):
                                rhs = x_rep16[
                                    :, :, r + a, 4 * j : 4 * j + 4
                                ].rearrange("p b g -> p g b")
                                nc.tensor.matmul(
                                    po[:, jh],
                                    tw[:, a],
                                    rhs,
                                    start=(a == 0),
                                    stop=(a == K - 1),
                                )
                        elif jh == 0:
                            nc.vector.memset(po[:, jh], 0.0)

                    # ReLU + extract diagonal blocks (g' == g) for the half-row
                    for g in range(4):
                        nc.scalar.activation(
                            ot[:, half, g],
                            po[32 * g : 32 * g + 32, :, g, :],
                            mybir.ActivationFunctionType.Relu,
                        )

                # per-row output DMA from ACT (overlaps later rows; only the
                # last row's DMA is an exposed tail)
                nc.scalar.dma_start(out=ov[:, :, r, :], in_=outT[:, :, r, :])

            if rep_ctx is not None:
                rep_ctx.__exit__(None, None, None)
    _split_multi_waits(nc)
    return nc


_NC_CACHE = None


def _in_maps(x: np.ndarray, weights: np.ndarray):
    xp = np.pad(x, ((0, 0), (0, 0), (1, 1), (1, 1)))  # [B, CIN, H+2, W+2]
    ident = np.eye(128, dtype=np.float32)
    maps = []
    for c in range(NCORES):
        h0 = c * HC
        maps.append(
            {
                "xs": np.ascontiguousarray(xp[:, :, h0 : h0 + HH, :]),
                "ws": np.ascontiguousarray(weights[h0 : h0 + HC]),
                "ident": ident,
            }
        )
    return maps


def kernel(x: np.ndarray, weights: np.ndarray) -> np.ndarray:
    global _NC_CACHE
    x = np.ascontiguousarray(x, dtype=np.float32)
    weights = np.ascontiguousarray(weights, dtype=np.float32)
    if _NC_CACHE is None:
        _NC_CACHE = _build_nc()
    res = run_bass_kernel_spmd(
        _NC_CACHE, _in_maps(x, weights), core_ids=list(range(NCORES))
    )
    out = np.concatenate([res.results[c]["out"] for c in range(NCORES)], axis=2)
    return np.ascontiguousarray(out, dtype=np.float32)


if __name__ == "__main__":
    rng = np.random.default_rng(0)
    x = rng.standard_normal((B, CIN, H, W), dtype=np.float32)
    w = rng.standard_normal((H, W, COUT, CIN, K, K), dtype=np.float32) / CIN
    y = kernel(x, w)
    print("out shape", y.shape, y.dtype)
